# revision 25
# baseline (speedup 1.0000x reference)
"""Mamba mixer (nn_Mixer) Trainium2 Bass kernel.

Sharding: 2-way data-parallel over batch x 4-way tensor-parallel over
d_inner.  Core k handles batch g=k//4 (all 4096 tokens) and d_inner
rows [512*(k%4), 512*(k%4+1)) (4 partition tiles of 128).

Pipeline per core (4 chunks of 1024 tokens):
  phase A (all chunks): in_proj (bf16 matmuls) -> causal conv1d+silu
    (DVE FIR taps) -> x_proj partial; xs/z spilled to DRAM.
  Two AllReduces over [96, 2048] bf16 x_dbl blocks (4-core groups
    [0-3],[4-7]).  Collectives execute in the Pool engine's in-order
    queue, so AR0 is emitted early (hides behind phase A of chunks
    2/3) and AR1 is emitted only after phase B of chunks 0/1 so their
    Pool-side scan work is not stuck behind it.
  phase B (per chunk): dt_proj + softplus -> selective scan.  B_n/C_n
    rows are partition-broadcast to [128, 1024] tiles by DMA (stride-0
    source, no compute engine), dA=exp(A*dt) on Act, dbx/h*C mults
    split DVE/Pool, the recurrence itself is the DVE hardware
    tensor_tensor_scan chained across chunks via a carry column per
    (dtile, state).  y = sum_n C_n*h_n accumulates over n in PSUM via
    an identity-stationary matmul, two dtiles at a time (psum budget:
    psA 4 banks + psY 4 banks).  Gate with silu(z), out_proj partial
    (row-parallel) -> bf16 -> DRAM; host sums the 4 partials per group
    (the unshard) and adds the D_skip*u residual.

Self-contained: hardcodes all shapes; only needs concourse.
"""

import os
import numpy as np

D_MODEL = 1024
D_INNER = 2048
NSTATE = 16
DT_RANK = 64
DCONV = 4
BATCH = 2
SEQ = 4096

NCORES = 8
TP = 4                       # d_inner shards per batch group
DS = D_INNER // TP           # 512 d_inner rows per core
DT4 = DS // 128              # 4 partition tiles per core
LC = 1024                    # chunk length (tokens)
NCH = SEQ // LC              # 4 chunks per core
NSUB = LC // 512             # 512-col psum subtiles per chunk
NXD = DT_RANK + 2 * NSTATE   # 96


def _build_nc(fake_collective=False, no_bcast=False):
    import concourse.bass as bass
    import concourse.bacc as bacc
    import concourse.mybir as mybir
    import concourse.tile as tile

    f32 = mybir.dt.float32
    bf16 = mybir.dt.bfloat16
    AF = mybir.ActivationFunctionType
    OP = mybir.AluOpType

    nc = bacc.Bacc("TRN2", target_bir_lowering=False, debug=False,
                   num_devices=NCORES)

    uT = nc.dram_tensor("uT16", [D_MODEL, SEQ], bf16, kind="ExternalInput")
    w_in = nc.dram_tensor("w_inT", [D_MODEL, 2 * DS], bf16, kind="ExternalInput")
    conv_w = nc.dram_tensor("conv_w", [128, DT4 * DCONV], f32, kind="ExternalInput")
    conv_b = nc.dram_tensor("conv_b", [128, DT4], f32, kind="ExternalInput")
    w_xp = nc.dram_tensor("w_xpT", [DS, NXD], bf16, kind="ExternalInput")
    w_dt = nc.dram_tensor("w_dtT", [DT_RANK, DS], bf16, kind="ExternalInput")
    dt_bias = nc.dram_tensor("dt_bias", [128, DT4], f32, kind="ExternalInput")
    a_neg = nc.dram_tensor("a_neg", [128, DT4 * NSTATE], f32, kind="ExternalInput")
    d_in = nc.dram_tensor("d_in", [128, DT4], f32, kind="ExternalInput")
    w_out = nc.dram_tensor("w_outT", [DS, D_MODEL], bf16, kind="ExternalInput")
    eye_d = nc.dram_tensor("eye128", [128, 128], bf16, kind="ExternalInput")
    y_part = nc.dram_tensor("y_part", [D_MODEL, SEQ], bf16, kind="ExternalOutput")

    with tile.TileContext(nc) as tc:
        with (
            tc.tile_pool(name="const", bufs=1) as cpool,
            tc.tile_pool(name="u", bufs=2) as upool,
            tc.tile_pool(name="x", bufs=1) as xpool,
            tc.tile_pool(name="za", bufs=2) as zpool,
            tc.tile_pool(name="da", bufs=1) as dapool,
            tc.tile_pool(name="cv", bufs=2) as cvpool,
            tc.tile_pool(name="bc", bufs=3) as bcpool,
            tc.tile_pool(name="nw", bufs=3) as npool,
            tc.tile_pool(name="g", bufs=2) as gpool,
            tc.tile_pool(name="yg", bufs=1) as ygpool,
            tc.tile_pool(name="psA", bufs=4, space="PSUM") as psA,
            tc.tile_pool(name="psY", bufs=1, space="PSUM") as psY,
            tc.tile_pool(name="dram", bufs=2, space="DRAM") as dpool,
        ):
            # ---- static weights into SBUF ----
            w_in_sb = cpool.tile([128, 8, 2 * DS], bf16)
            nc.sync.dma_start(w_in_sb[:], w_in.ap().rearrange(
                "(j p) m -> p j m", p=128))
            w_out_sb = cpool.tile([128, DT4, D_MODEL], bf16)
            nc.sync.dma_start(w_out_sb[:], w_out.ap().rearrange(
                "(k p) m -> p k m", p=128))
            w_xp_sb = cpool.tile([128, DT4, NXD], bf16)
            nc.sync.dma_start(w_xp_sb[:], w_xp.ap().rearrange(
                "(k p) m -> p k m", p=128))
            w_dt_sb = cpool.tile([DT_RANK, DS], bf16)
            nc.sync.dma_start(w_dt_sb[:], w_dt.ap())
            conv_w_sb = cpool.tile([128, DT4 * DCONV], f32)
            nc.sync.dma_start(conv_w_sb[:], conv_w.ap())
            conv_b_sb = cpool.tile([128, DT4], f32)
            nc.sync.dma_start(conv_b_sb[:], conv_b.ap())
            dt_bias_sb = cpool.tile([128, DT4], f32)
            nc.sync.dma_start(dt_bias_sb[:], dt_bias.ap())
            a_sb = cpool.tile([128, DT4 * NSTATE], f32)
            nc.sync.dma_start(a_sb[:], a_neg.ap())
            d_in_sb = cpool.tile([128, DT4], f32)
            nc.sync.dma_start(d_in_sb[:], d_in.ap())
            eye16 = cpool.tile([128, 128], bf16)
            nc.sync.dma_start(eye16[:], eye_d.ap())
            carry = cpool.tile([128, DT4 * NSTATE], f32)
            nc.vector.memset(carry[:], 0.0)
            halo = cpool.tile([128, DT4, DCONV - 1], bf16)

            uT_ap = uT.ap().rearrange("(j p) t -> p j t", p=128)

            # DRAM staging: xs/z spill per chunk, and 2 batched AllReduces
            xs_dram = dpool.tile([128, NCH, DT4, LC], bf16, tag="xsd")
            z_dram = dpool.tile([128, NCH, DT4, LC], bf16, tag="zd")
            ar_in = dpool.tile([2, NXD, 2 * LC], bf16, tag="arin")
            ar_out = dpool.tile([2, NXD, 2 * LC], bf16, tag="arout")

            # ---- phase A / phase B bodies ----
            def phase_a(c):
                t0 = c * LC
                x_sb = xpool.tile([128, DT4, LC + DCONV - 1], bf16, tag="x")
                z_sil = zpool.tile([128, DT4, LC], bf16, tag="z")
                for sub in range(NSUB):
                    ts = t0 + 512 * sub
                    u_sb = upool.tile([128, 8, 512], bf16, tag="u")
                    nc.sync.dma_start(u_sb[:], uT_ap[:, :, ts:ts + 512])
                    for mt in range(2 * DT4):
                        ps = psA.tile([128, 512], f32, tag="mm")
                        for j in range(8):
                            nc.tensor.matmul(
                                ps[:], w_in_sb[:, j, 128 * mt:128 * (mt + 1)],
                                u_sb[:, j, :], start=(j == 0), stop=(j == 7))
                        if mt < DT4:
                            nc.scalar.copy(
                                x_sb[:, mt, DCONV - 1 + 512 * sub:
                                     DCONV - 1 + 512 * (sub + 1)], ps[:])
                        else:
                            nc.scalar.activation(
                                z_sil[:, mt - DT4, 512 * sub:512 * (sub + 1)],
                                ps[:], AF.Silu, bias=0.0)
                nc.sync.dma_start(z_dram[:, c, :, :], z_sil[:])

                # conv halo in, save tail for next chunk
                for dt in range(DT4):
                    if c == 0:
                        nc.gpsimd.memset(x_sb[:, dt, 0:DCONV - 1], 0.0)
                    else:
                        nc.gpsimd.tensor_copy(x_sb[:, dt, 0:DCONV - 1],
                                              halo[:, dt, :])

                # ---- causal conv1d + silu, spill xs ----
                xs = zpool.tile([128, DT4, LC], bf16, tag="xs")
                for dt in range(DT4):
                    ca = cvpool.tile([128, LC], bf16, tag="ca")
                    cb = cvpool.tile([128, LC], bf16, tag="cb")
                    nc.vector.tensor_scalar_mul(
                        ca[:], x_sb[:, dt, 0:LC],
                        conv_w_sb[:, dt * DCONV:dt * DCONV + 1])
                    src, dst = ca, cb
                    for k in range(1, DCONV):
                        nc.vector.scalar_tensor_tensor(
                            dst[:], x_sb[:, dt, k:k + LC],
                            conv_w_sb[:, dt * DCONV + k:dt * DCONV + k + 1],
                            src[:], op0=OP.mult, op1=OP.add)
                        src, dst = dst, src
                    nc.gpsimd.tensor_copy(halo[:, dt, :],
                                          x_sb[:, dt, LC:LC + DCONV - 1])
                    nc.scalar.activation(xs[:, dt, :], src[:], AF.Silu,
                                         bias=conv_b_sb[:, dt:dt + 1])
                nc.sync.dma_start(xs_dram[:, c, :, :], xs[:])

                # ---- x_proj partial -> ar_in ----
                xd16 = gpool.tile([NXD, LC], bf16, tag="xd16")
                for sub in range(NSUB):
                    ps = psA.tile([128, 512], f32, tag="mm")
                    for dt in range(DT4):
                        nc.tensor.matmul(ps[0:NXD, :], w_xp_sb[:, dt, :],
                                         xs[:, dt, 512 * sub:512 * (sub + 1)],
                                         start=(dt == 0), stop=(dt == DT4 - 1))
                    nc.scalar.copy(xd16[:, 512 * sub:512 * (sub + 1)],
                                   ps[0:NXD, :])
                nc.sync.dma_start(
                    ar_in[c // 2, :, (c % 2) * LC:(c % 2) * LC + LC], xd16[:])


            # ---- phase B body ----
            def phase_b(c):
                t0 = c * LC
                xs = zpool.tile([128, DT4, LC], bf16, tag="xs2")
                nc.sync.dma_start(xs[:], xs_dram[:, c, :, :])
                z_sil = zpool.tile([128, DT4, LC], bf16, tag="z2")
                nc.sync.dma_start(z_sil[:], z_dram[:, c, :, :])
                xdt = gpool.tile([DT_RANK, LC], bf16, tag="xdt")
                co = (c % 2) * LC
                nc.sync.dma_start(xdt[:],
                                  ar_out[c // 2, 0:DT_RANK, co:co + LC])

                # ---- dt = softplus(dt_proj @ xdt + bias) ----
                dt_sb = dapool.tile([128, DT4, LC], bf16, tag="dt")
                for dt in range(DT4):
                    e_t = npool.tile([128, LC], bf16, tag="esp")
                    for sub in range(NSUB):
                        ps = psA.tile([128, 512], f32, tag="mm")
                        nc.tensor.matmul(ps[:],
                                         w_dt_sb[:, 128 * dt:128 * (dt + 1)],
                                         xdt[:, 512 * sub:512 * (sub + 1)],
                                         start=True, stop=True)
                        nc.scalar.activation(e_t[:, 512 * sub:512 * (sub + 1)],
                                             ps[:], AF.Exp,
                                             bias=dt_bias_sb[:, dt:dt + 1])
                    nc.scalar.activation(dt_sb[:, dt, :], e_t[:], AF.Ln,
                                         bias=1.0)

                dtx = dapool.tile([128, DT4, LC], bf16, tag="dtx")
                for dt in range(DT4):
                    nc.vector.tensor_mul(dtx[:, dt, :], dt_sb[:, dt, :],
                                         xs[:, dt, :])

                # ---- scan: dt-pairs, DMA-broadcast B/C ----
                yg = ygpool.tile([128, DT4, LC], bf16, tag="yg")
                for pair in range(2):
                    y_ps = psY.tile([128, 2, LC], f32, tag="y",
                                    name=f"yps{c}_{pair}")
                    for n in range(NSTATE):
                        bb = bcpool.tile([128, LC], bf16, tag="bb")
                        cc = bcpool.tile([128, LC], bf16, tag="cc")
                        nc.sync.dma_start(
                            bb[:], ar_out[c // 2, DT_RANK + n:DT_RANK + n + 1,
                                          co:co + LC].partition_broadcast(128))
                        nc.sync.dma_start(
                            cc[:], ar_out[c // 2, DT_RANK + NSTATE + n:
                                          DT_RANK + NSTATE + n + 1,
                                          co:co + LC].partition_broadcast(128))
                        for dt in (2 * pair, 2 * pair + 1):
                            col = dt * NSTATE + n
                            a_t = npool.tile([128, LC], bf16, tag="a")
                            nc.scalar.activation(
                                a_t[:], dt_sb[:, dt, :], AF.Exp,
                                bias=0.0, scale=a_sb[:, col:col + 1])
                            dbx = npool.tile([128, LC], bf16, tag="dbx")
                            if dt % 2 == 0:
                                nc.vector.tensor_mul(dbx[:], dtx[:, dt, :],
                                                     bb[:])
                            else:
                                nc.gpsimd.tensor_mul(dbx[:], dtx[:, dt, :],
                                                     bb[:])
                            h_t = npool.tile([128, LC], bf16, tag="h")
                            nc.vector.tensor_tensor_scan(
                                h_t[:], a_t[:], dbx[:],
                                initial=carry[:, col:col + 1],
                                op0=OP.mult, op1=OP.add)
                            nc.gpsimd.tensor_copy(carry[:, col:col + 1],
                                                  h_t[:, LC - 1:LC])
                            w_t = npool.tile([128, LC], bf16, tag="w")
                            if dt % 2 == 0:
                                nc.vector.tensor_mul(w_t[:], h_t[:], cc[:])
                            else:
                                nc.gpsimd.tensor_mul(w_t[:], h_t[:], cc[:])
                            for sub in range(NSUB):
                                nc.tensor.matmul(
                                    y_ps[:, dt - 2 * pair,
                                         512 * sub:512 * (sub + 1)],
                                    eye16[:],
                                    w_t[:, 512 * sub:512 * (sub + 1)],
                                    start=(n == 0), stop=(n == NSTATE - 1))

                    for dt in (2 * pair, 2 * pair + 1):
                        ys = npool.tile([128, LC], bf16, tag="ys")
                        nc.vector.scalar_tensor_tensor(
                            ys[:], xs[:, dt, :], d_in_sb[:, dt:dt + 1],
                            y_ps[:, dt - 2 * pair, :], op0=OP.mult,
                            op1=OP.add)
                        nc.vector.tensor_mul(yg[:, dt, :], ys[:],
                                             z_sil[:, dt, :])

                # ---- out_proj partial -> bf16 -> DRAM ----
                for mt in range(8):
                    for sub in range(NSUB):
                        ps = psA.tile([128, 512], f32, tag="mm")
                        for kt in range(DT4):
                            nc.tensor.matmul(
                                ps[:], w_out_sb[:, kt, 128 * mt:128 * (mt + 1)],
                                yg[:, kt, 512 * sub:512 * (sub + 1)],
                                start=(kt == 0), stop=(kt == DT4 - 1))
                        ob = gpool.tile([128, 512], bf16, tag="ob")
                        if mt % 2 == 0:
                            nc.scalar.copy(ob[:], ps[:])
                        else:
                            nc.vector.tensor_copy(ob[:], ps[:])
                        nc.sync.dma_start(
                            y_part[128 * mt:128 * (mt + 1),
                                   t0 + 512 * sub:t0 + 512 * (sub + 1)],
                            ob[:])

            def all_reduce(h):
                if fake_collective:
                    nc.sync.dma_start(ar_out[h], ar_in[h])
                else:
                    nc.gpsimd.collective_compute(
                        "AllReduce", OP.add,
                        replica_groups=[[0, 1, 2, 3], [4, 5, 6, 7]],
                        ins=[ar_in[h].opt()], outs=[ar_out[h].opt()])

            # schedule: AR0 early (gates B0/B1); AR1 after B0/B1 are
            # emitted so their Pool-queue work is not stuck behind it.
            phase_a(0)
            phase_a(1)
            all_reduce(0)
            phase_a(2)
            phase_a(3)
            phase_b(0)
            phase_b(1)
            all_reduce(1)
            phase_b(2)
            phase_b(3)

    nc.compile()
    return nc


_CACHED = {}


def _get_nc():
    if "nc" not in _CACHED:
        fake = bool(int(os.environ.get("MAMBA_FAKE_AR", "0")))
        nb = bool(int(os.environ.get("MAMBA_NO_BCAST", "0")))
        _CACHED["nc"] = _build_nc(fake_collective=fake, no_bcast=nb)
    return _CACHED["nc"]


def _host_prep(inputs):
    import ml_dtypes
    _bf = ml_dtypes.bfloat16
    f32 = np.float32
    u = np.asarray(inputs["u"], f32)
    in_proj_w = np.asarray(inputs["in_proj_w"], f32)
    conv_w = np.asarray(inputs["conv_w"], f32)
    conv_b = np.asarray(inputs["conv_b"], f32)
    x_proj_w = np.asarray(inputs["x_proj_w"], f32)
    dt_proj_w = np.asarray(inputs["dt_proj_w"], f32)
    dt_bias = np.asarray(inputs["dt_bias"], f32)
    A_log = np.asarray(inputs["A_log"], f32)
    D_in = np.asarray(inputs["D_in"], f32)
    out_proj_w = np.asarray(inputs["out_proj_w"], f32)

    eye = np.eye(128, dtype=f32).astype(_bf)
    A = -np.exp(A_log)

    # uT per batch group, bf16
    uTs = [np.ascontiguousarray(u[g].T).astype(_bf) for g in range(BATCH)]

    def fold(v):  # (512, k) -> (128, 4*k) dtile-major columns
        v = v.reshape(DS, -1)
        return np.ascontiguousarray(np.concatenate(
            [v[128 * i:128 * (i + 1)] for i in range(DT4)], axis=1))

    in_maps = []
    for k in range(NCORES):
        g, r = divmod(k, TP)
        sl = slice(DS * r, DS * (r + 1))
        w_in_k = np.concatenate(
            [in_proj_w[sl],
             in_proj_w[D_INNER + DS * r:D_INNER + DS * (r + 1)]])
        in_maps.append({
            "uT16": uTs[g],
            "w_inT": np.ascontiguousarray(w_in_k.T).astype(_bf),
            "conv_w": fold(conv_w[sl]),
            "conv_b": fold(conv_b[sl]),
            "w_xpT": np.ascontiguousarray(x_proj_w[:, sl].T).astype(_bf),
            "w_dtT": np.ascontiguousarray(dt_proj_w[sl].T).astype(_bf),
            "dt_bias": fold(dt_bias[sl]),
            "a_neg": fold(A[sl]),
            "d_in": fold(D_in[sl]),
            "w_outT": np.ascontiguousarray(out_proj_w[:, sl].T).astype(_bf),
            "eye128": eye,
        })
    return in_maps


LAST_RESULTS = None


def bench(inputs, iters=24, warmup=4):
    """Marginal per-execution device time of the jitted NEFF."""
    import time
    import jax
    from jax.sharding import Mesh, PartitionSpec, NamedSharding
    from jax.experimental.shard_map import shard_map
    import concourse.mybir as mybir
    from concourse import bass2jax
    from concourse.bass2jax import _bass_exec_p, install_neuronx_cc_hook

    install_neuronx_cc_hook()
    nc = _get_nc()
    in_maps = _host_prep(inputs)

    partition_name = (nc.partition_id_tensor.name
                      if nc.partition_id_tensor else None)
    in_names, out_names, out_avals, zero_outs = [], [], [], []
    for alloc in nc.m.functions[0].allocations:
        if not isinstance(alloc, mybir.MemoryLocationSet):
            continue
        name = alloc.memorylocations[0].name
        if alloc.kind == "ExternalInput":
            if name != partition_name:
                in_names.append(name)
        elif alloc.kind == "ExternalOutput":
            shape = tuple(alloc.tensor_shape)
            dtype = mybir.dt.np(alloc.dtype)
            out_avals.append(jax.core.ShapedArray(shape, dtype))
            out_names.append(name)
            zero_outs.append(np.zeros(shape, dtype))
    n_params = len(in_names)
    all_in_names = list(in_names) + list(out_names)
    if partition_name is not None:
        all_in_names.append(partition_name)

    def _body(*args):
        operands = list(args)
        if partition_name is not None:
            operands.append(bass2jax.partition_id_tensor())
        outs = _bass_exec_p.bind(
            *operands,
            out_avals=tuple(out_avals),
            in_names=tuple(all_in_names),
            out_names=tuple(out_names),
            lowering_input_output_aliases=(),
            sim_require_finite=True,
            sim_require_nnan=True,
            nc=nc,
        )
        return tuple(outs)

    devices = jax.devices()[:NCORES]
    mesh = Mesh(np.asarray(devices), ("core",))
    in_specs = (PartitionSpec("core"),) * (n_params + len(out_names))
    out_specs = (PartitionSpec("core"),) * len(out_names)
    fn = jax.jit(shard_map(_body, mesh=mesh, in_specs=in_specs,
                           out_specs=out_specs, check_rep=False),
                 keep_unused=True)

    concat_in = [np.concatenate([in_maps[c][nm] for c in range(NCORES)],
                                axis=0) for nm in in_names]
    concat_zeros = [np.zeros((NCORES * z.shape[0], *z.shape[1:]), z.dtype)
                    for z in zero_outs]
    sh = NamedSharding(mesh, PartitionSpec("core"))
    dev_in = [jax.device_put(a, sh) for a in concat_in + concat_zeros]

    for _ in range(warmup):
        outs = fn(*dev_in)
    jax.block_until_ready(outs)
    times = {}
    for it in (iters // 4, iters):
        t0 = time.perf_counter()
        for _ in range(it):
            outs = fn(*dev_in)
        jax.block_until_ready(outs)
        times[it] = time.perf_counter() - t0
    ks = sorted(times)
    return (times[ks[1]] - times[ks[0]]) / (ks[1] - ks[0])


def kernel(**inputs):
    global LAST_RESULTS
    from concourse import bass_utils

    u = np.asarray(inputs["u"], np.float32)
    D_skip = np.asarray(inputs["D_skip"], np.float32)

    nc = _get_nc()
    in_maps = _host_prep(inputs)
    trace = bool(int(os.environ.get("MAMBA_TRACE", "0")))
    res = bass_utils.run_bass_kernel_spmd(
        nc, in_maps, core_ids=list(range(NCORES)), trace=trace)
    LAST_RESULTS = res

    y = np.zeros((BATCH, SEQ, D_MODEL), np.float32)
    for k, r in enumerate(res.results):
        g = k // TP
        y[g] += np.asarray(r["y_part"]).astype(np.float32).T
    return y + D_skip[None, None, :] * u
